# revision 101
# baseline (speedup 1.0000x reference)
"""Trainium2 Bass kernel for RangeLinearQuantParamLayerWrapper (symmetric int8
quantized linear: y = dequant(requant(x_q @ W_q.T + b_q))).

Full inputs in, full output out. Sharded over 8 NeuronCores on a
4 (batch) x 2 (out_features) grid:
  - x^T shard  [4096 i, 1024 b]  per core (batch cols)
  - W^T shard  [16 m-blocks, 4096 i, 128 o] per core (out cols)
  - global max-abs scan is split so each core reads a disjoint 1/8 of x and
    of W: per core, x k-tiles [0:16) of its shard (k-rolled per out-group so
    the union covers all k, all 16 staged in SBUF and reused by the
    quantizer) and W m-blocks [0:4) of its shard (m-permuted per batch-group
    so the union covers all m; re-read later, DMA has slack under the PE)
  - ONE AllReduce(max) over [gx, gw, gb] for all input scales
  - junk warmup matmuls riding the scale chain keep the PE p-state warm
  - bf16 matmul (quantized values are exact small ints in bf16), f32 PSUM,
    m-blocks in groups [3,2,2,2,2,2,2,1] with k outermost inside a group so
    PSUM banks double-buffer and the PE never stalls
  - second AllReduce(max) over the int32 accumulator for the output scale
  - output leaves the device as int8 out_q plus the scalar 1/out_scale;
    the host applies the same IEEE f32 dequant multiply during unshard
Output per core: out_q^T block [2048 o, 1024 b], transposed+assembled on host.
"""
import sys

sys.path.insert(0, "/opt/trn_rl_repo")
import numpy as np

NCORES = 8
GR, GC = 4, 2          # core grid: 4 batch groups x 2 out-feature groups
B = O = K = 4096
BS = B // GR           # 1024 batch cols per core
OS = O // GC           # 2048 out cols per core
MT = OS // 128         # 16 o-blocks per core
KT = K // 128          # 32 k tiles
NSTW = MT // GR        # 4 absmax-scanned W m-blocks per core
NSTX = KT // GC        # 16 absmax-scanned x k-tiles per core (all staged)
GROUPS = [3, 2, 2, 2, 2, 2, 2, 1]
C_MAGIC = 1.5 * 2.0**23          # f32 add/sub constant => round-to-nearest-even
C_P127 = C_MAGIC + 127.0

_CACHE = {}


def _build_nc(sim_single_core=False):
    import concourse.bass as bass
    import concourse.mybir as mybir
    import concourse.tile as tile
    from concourse import bacc, bass_isa

    f32 = mybir.dt.float32
    bf16 = mybir.dt.bfloat16
    Alu = mybir.AluOpType
    Act = mybir.ActivationFunctionType

    nc = bacc.Bacc("TRN2", target_bir_lowering=False, debug=False,
                   num_devices=1 if sim_single_core else NCORES)

    def all_reduce_max(cin_ap, cout_ap):
        if sim_single_core:
            return nc.sync.dma_start(cout_ap, cin_ap)
        return nc.gpsimd.collective_compute(
            "AllReduce", mybir.AluOpType.max,
            replica_groups=[list(range(NCORES))],
            ins=[cin_ap.opt()], outs=[cout_ap.opt()])

    i8 = mybir.dt.int8
    xt_d = nc.dram_tensor("xt", [K, BS], f32, kind="ExternalInput")
    wt_d = nc.dram_tensor("wt", [MT, K, 128], f32, kind="ExternalInput")
    bp_d = nc.dram_tensor("bp", [128, 32], f32, kind="ExternalInput")
    # output stays quantized (int8) on device; the trivial dequant
    # multiply by the scalar 1/out_scale happens on the host during
    # unshard (bit-identical f32 multiply, 4x less DMA + PCIe traffic)
    out_d = nc.dram_tensor("out", [OS, BS], i8, kind="ExternalOutput")
    ios_d = nc.dram_tensor("ios", [1, 4], f32, kind="ExternalOutput")

    with tile.TileContext(nc) as tc:
        with (
            tc.tile_pool(name="pers", bufs=1) as pers,
            tc.tile_pool(name="psum", bufs=8, space="PSUM") as psum,
            tc.tile_pool(name="dram", bufs=1, space="DRAM") as dram,
            tc.tile_pool(name="stat", bufs=3) as stat,
        ):
            # staged pool: x tiles the absmax pass reads stay in SBUF
            xst_ctx = tc.tile_pool(name="xstp", bufs=1)
            xst = xst_ctx.__enter__()
            wab_ctx = tc.tile_pool(name="wabp", bufs=3)
            wab = wab_ctx.__enter__()

            # ------------- phase A: local absmax of this core's 1/8 -------
            mx = pers.tile([128, 1], f32, tag="mx")
            mw = pers.tile([128, 1], f32, tag="mw")
            nc.vector.memset(mx[:], 0.0)
            nc.vector.memset(mw[:], 0.0)

            # tiny warmup activation so the ACT function-table load (1.3us)
            # happens at t=0 instead of blocking the first W quantize
            awarm = pers.tile([1, 1], f32, tag="awarm")
            nc.scalar.activation(awarm[:], mx[0:1, 0:1], Act.Identity,
                                 bias=0.0, scale=1.0)

            # W m-blocks 0..NSTW-1 first, streamed; DVE scans halves
            # (quant re-reads W later - DMA has slack under the matmul)
            for m in range(NSTW):
                wm = wab.tile([128, KT, 128], f32, tag="wab")
                nc.sync.dma_start(
                    wm[:], wt_d.ap()[m].rearrange("(k p) j -> p k j", p=128))
                for h in range(2):
                    rw = stat.tile([128, 1], f32, tag="absw")
                    nc.vector.tensor_reduce(
                        rw[:], wm[:, 16 * h:16 * (h + 1), :],
                        axis=mybir.AxisListType.XY, op=Alu.max,
                        apply_absolute_value=True)
                    nc.vector.tensor_max(mw[:], mw[:], rw[:])
            wab_ctx.__exit__(None, None, None)

            bp = pers.tile([128, 32], f32, tag="bp")
            nc.sync.dma_start(bp[:], bp_d.ap())
            mb = pers.tile([128, 1], f32, tag="mb")
            nc.vector.tensor_reduce(
                mb[:], bp[:], axis=mybir.AxisListType.X, op=Alu.max,
                apply_absolute_value=True)

            # AllReduce input lanes [gx, gw, gb]; w/b lanes fill early,
            # the x lane is written by the last scan reduce directly
            stk = pers.tile([128, 4], f32, tag="stk")
            nc.vector.tensor_copy(stk[:, 1:2], mw[:])
            nc.vector.tensor_copy(stk[:, 2:3], mb[:])
            nc.vector.tensor_copy(stk[:, 3:4], mb[:])

            # all NSTX scanned x k-tiles staged via single-tile DMAs so each
            # DVE reduce chases its own DMA - the scan ends ~1us after the
            # last DMA lands
            xstage = []
            for k in range(NSTX):
                xtile = xst.tile([128, BS], f32, tag=f"xst{k}")
                if k < NSTX - 1:
                    nc.sync.dma_start(
                        xtile[:], xt_d.ap()[128 * k:128 * (k + 1), :])
                    rt = stat.tile([128, 1], f32, tag="absx")
                    nc.vector.tensor_reduce(
                        rt[:], xtile[:], axis=mybir.AxisListType.X,
                        op=Alu.max, apply_absolute_value=True)
                    nc.vector.tensor_max(mx[:], mx[:], rt[:])
                else:
                    # last scanned tile in quarters: its reduce is the gate
                    # for the scale AllReduce, so minimize the chase lag
                    for h in range(4):
                        nc.sync.dma_start(
                            xtile[:, 256 * h:256 * (h + 1)],
                            xt_d.ap()[128 * k:128 * (k + 1),
                                      256 * h:256 * (h + 1)])
                        rt = stat.tile([128, 1], f32, tag="absx")
                        nc.vector.tensor_reduce(
                            rt[:], xtile[:, 256 * h:256 * (h + 1)],
                            axis=mybir.AxisListType.X,
                            op=Alu.max, apply_absolute_value=True)
                        if h < 3:
                            nc.vector.tensor_max(mx[:], mx[:], rt[:])
                        else:
                            nc.vector.tensor_max(stk[:, 0:1], mx[:], rt[:])
                xstage.append(xtile)
            par = pers.tile([128, 4], f32, tag="par")
            nc.gpsimd.partition_all_reduce(
                par[:], stk[:], channels=128, reduce_op=bass_isa.ReduceOp.max)
            cin = dram.tile([1, 8], f32, tag="cin")
            cout = dram.tile([1, 8], f32, tag="cout")
            nc.sync.dma_start(cin[0:1, 0:4], par[0:1, 0:4])
            ar1 = all_reduce_max(cin[0:1, 0:4], cout[0:1, 0:4])
            gm = pers.tile([1, 8], f32, tag="gm")
            gmd = nc.sync.dma_start(gm[:], cout[:])

            # scalars: s = 255/(2*g) = 127.5*(1/g) for [sx, sw, sb];
            # asc = sx*sw; fb = asc/sb; rac = 1/asc (for the output stage)
            rc = pers.tile([1, 4], f32, tag="rc")
            nc.vector.reciprocal(rc[0:1, 0:3], gm[0:1, 0:3])
            scal = pers.tile([1, 4], f32, tag="scal")
            nc.vector.tensor_scalar(out=scal[0:1, 0:3], in0=rc[0:1, 0:3],
                                    scalar1=127.5, scalar2=None, op0=Alu.mult)
            sx, sw, sb = scal[0:1, 0:1], scal[0:1, 1:2], scal[0:1, 2:3]
            asc = pers.tile([1, 1], f32, tag="asc")        # accum_scale
            nc.vector.tensor_mul(asc[:], sx, sw)
            rbs = pers.tile([1, 1], f32, tag="rbs")
            nc.vector.reciprocal(rbs[:], sb)
            nc.vector.tensor_mul(scal[0:1, 3:4], asc[:], rbs[:])  # fb
            rac = pers.tile([1, 1], f32, tag="rac")
            nc.vector.reciprocal(rac[:], asc[:])
            c2 = pers.tile([1, 1], f32, tag="c2")  # 1/(127.5*asc), for ios
            nc.vector.tensor_scalar(out=c2[:], in0=rac[:],
                                    scalar1=1.0 / 127.5, scalar2=None,
                                    op0=Alu.mult)
            scb = pers.tile([128, 4], f32, tag="scb")
            nc.gpsimd.partition_broadcast(scb[:], scal[:], channels=128)

            cbias = pers.tile([128, 1], f32, tag="cbias")
            nc.vector.memset(cbias[:], C_MAGIC)

            # ---------------- b quantize ----------------
            bq1 = pers.tile([128, 32], f32, tag="bq1")
            nc.vector.tensor_scalar(out=bq1[:], in0=bp[:],
                                    scalar1=scb[:, 2:3], scalar2=C_MAGIC,
                                    op0=Alu.mult, op1=Alu.add)
            bq2 = pers.tile([128, 32], f32, tag="bq2")
            nc.vector.tensor_scalar(out=bq2[:], in0=bq1[:],
                                    scalar1=C_MAGIC, scalar2=127.0,
                                    op0=Alu.subtract, op1=Alu.min)
            bq3 = pers.tile([128, 16], f32, tag="bq3")
            nc.vector.tensor_scalar(out=bq3[:], in0=bq2[:, 0:16],
                                    scalar1=scb[:, 3:4], scalar2=C_MAGIC,
                                    op0=Alu.mult, op1=Alu.add)
            bqf = pers.tile([128, 16], f32, tag="bqf")
            nc.vector.tensor_scalar(out=bqf[:], in0=bq3[:],
                                    scalar1=C_MAGIC, scalar2=None,
                                    op0=Alu.subtract)

            # PE warmup: tiny junk matmuls gated on the scale chain's last
            # DMA keep the PE continuously busy from mid-chain until the
            # real stream starts, so the p-state ramp is paid on junk work
            from concourse.bass import _add_dep_helper
            jw = pers.tile([128, 128], bf16, tag="jw")
            nc.vector.memset(jw[:], 0.0)
            wps = psum.tile([128, 512], f32, tag="ps", name="wps")
            for i in range(90):
                mm = nc.tensor.matmul(wps[:, 0:64], jw[:, 0:128], jw[:, 0:64],
                                      start=True, stop=True)
                if i == 0:
                    _add_dep_helper(mm.ins, gmd.ins,
                                    reason="warmup rides the scale chain")

            # ---------------- quantize + matmul ----------------
            accs = []
            macc = pers.tile([128, 1], f32, tag="macc")
            xq_ctx = tc.tile_pool(name="xqp", bufs=1, side="right")
            xqp = xq_ctx.__enter__()
            with (
                tc.tile_pool(name="wsp", bufs=3, side="right") as wsp,
                tc.tile_pool(name="wt1p", bufs=2, side="right") as wt1p,
                tc.tile_pool(name="wqp", bufs=5, side="right") as wqp,
            ):
                xs_ctx = tc.tile_pool(name="xsp", bufs=2, side="right")
                xsp = xs_ctx.__enter__()
                xt1_ctx = tc.tile_pool(name="xt1p", bufs=2, side="right")
                xt1p = xt1_ctx.__enter__()

                # x quant: all on DVE (ACT/Pool are reserved for W)
                xq = [None] * KT

                def quant_x(k):
                    if k < NSTX:
                        xs = xstage[k][:]
                    else:
                        xt = xsp.tile([128, BS], f32, tag="xs")
                        xd = nc.sync.dma_start(
                            xt[:], xt_d.ap()[128 * k:128 * (k + 1), :])
                        _dep(xd)
                        xs = xt[:]
                    xt1 = xt1p.tile([128, BS], f32, tag="xt1")
                    nc.vector.tensor_scalar(out=xt1[:], in0=xs,
                                            scalar1=scb[:, 0:1],
                                            scalar2=C_MAGIC,
                                            op0=Alu.mult, op1=Alu.add)
                    xqk = xqp.tile([128, BS], bf16, tag=f"xq{k}")
                    nc.vector.tensor_scalar(out=xqk[:], in0=xt1[:],
                                            scalar1=C_MAGIC, scalar2=127.0,
                                            op0=Alu.subtract, op1=Alu.min)
                    xq[k] = xqk

                # quant-phase DMAs wait for the scale chain's last DMA so
                # the tiny chain hops aren't queued behind their transfers
                def _dep(dma):
                    _add_dep_helper(dma.ins, gmd.ins,
                                    reason="scale chain DMAs go first")

                # W quant in quarter-blocks:
                # DMA -> ACT (scale+magic) -> Pool/DVE (unmagic+clip) -> bf16
                def quant_w_quarter(m, q, wqm, fast=False):
                    src = wt_d.ap()[m].rearrange("(k p) j -> p k j", p=128)
                    ws = wsp.tile([128, 8, 128], f32, tag="ws")
                    wd = nc.sync.dma_start(
                        ws[:], src[:, 8 * q:8 * (q + 1), :])
                    if fast:
                        # ride right behind the AllReduce so the data is
                        # in SBUF the moment the scale broadcast lands
                        _add_dep_helper(wd.ins, ar1.ins,
                                        reason="first quarters ride the AR")
                    elif m < 8:
                        _dep(wd)
                    # fast path: first k-slices tiny, TS2 on DVE (skips a
                    # cross-engine hop) so the PE starts earliest
                    if fast:
                        subs, eng2 = ((0, 2), (2, 8)), nc.vector
                    else:
                        subs, eng2 = ((0, 8),), nc.gpsimd
                    wt1 = wt1p.tile([128, 8, 128], f32, tag="wt1q")
                    for a, b in subs:
                        nc.scalar.activation(wt1[:, a:b, :],
                                             ws[:, a:b, :], Act.Identity,
                                             bias=cbias[:, 0:1],
                                             scale=scb[:, 1:2])
                        eng2.tensor_scalar(
                            out=wqm[:, 8 * q + a:8 * q + b, :],
                            in0=wt1[:, a:b, :],
                            scalar1=C_MAGIC, scalar2=127.0,
                            op0=Alu.subtract, op1=Alu.min)

                def quant_w(m):
                    wqm = wqp.tile([128, KT, 128], bf16, tag="wq")
                    for q in range(4):
                        quant_w_quarter(m, q, wqm)
                    return wqm

                # group 0's blocks quantize with quarters interleaved
                # across blocks so all three k=0 slices arrive early and
                # the PE starts dense (sparse starts reset its p-state)
                quant_x(0)
                wq_pipe = [wqp.tile([128, KT, 128], bf16, tag="wq",
                                    name=f"wq_g0_{i}")
                           for i in range(GROUPS[0])]
                quant_w_quarter(0, 0, wq_pipe[0], fast=True)
                quant_x(1)
                quant_w_quarter(1, 0, wq_pipe[1], fast=True)
                quant_x(2)
                quant_w_quarter(2, 0, wq_pipe[2])
                for q in range(1, 4):
                    for mi in range(GROUPS[0]):
                        quant_w_quarter(mi, q, wq_pipe[mi])
                for k in range(3, KT):
                    quant_x(k)
                xt1_ctx.__exit__(None, None, None)
                xs_ctx.__exit__(None, None, None)
                xst_ctx.__exit__(None, None, None)
                acc_ctx = tc.tile_pool(name="accp", bufs=1)
                accp = acc_ctx.__enter__()

                # m-blocks in groups, k outermost within a group: each xq[k]
                # feeds 2*gsz matmuls; groups of 2 double-buffer PSUM banks
                m0 = 0
                for gi, gsz in enumerate(GROUPS):
                    if gi + 1 < len(GROUPS):
                        for j in range(GROUPS[gi + 1]):
                            wq_pipe.append(quant_w(m0 + gsz + j))
                    gacc = [accp.tile([128, BS], f32, tag=f"acc{m0 + i}",
                                      name=f"acc{m0 + i}")
                            for i in range(gsz)]
                    ps = [psum.tile([128, 512], f32, tag="ps",
                                    name=f"ps{gi}_{i}")
                          for i in range(2 * gsz)]
                    if m0 == MT - 1:
                        # last block: finish bank n=1 completely first so
                        # its PSUM copy + reduce overlap bank n=0's matmuls
                        # (the accum-max AllReduce is the critical path)
                        for n in (1, 0):
                            for k in range(KT):
                                nc.tensor.matmul(
                                    ps[n][:], wq_pipe[m0][:, k, :],
                                    xq[k][:, 512 * n:512 * (n + 1)],
                                    start=(k == 0), stop=(k == KT - 1))
                    else:
                        for k in range(KT):
                            for mi in range(gsz):
                                wq_cur = wq_pipe[m0 + mi]
                                for n in range(2):
                                    nc.tensor.matmul(
                                        ps[2 * mi + n][:], wq_cur[:, k, :],
                                        xq[k][:, 512 * n:512 * (n + 1)],
                                        start=(k == 0), stop=(k == KT - 1))
                    for mi in range(gsz):
                        acc_m = gacc[mi]
                        if m0 + mi == MT - 1:
                            # last block: PSUM copies on two engines in
                            # parallel (this is the AR2 critical path)
                            nc.scalar.activation(
                                acc_m[:, 0:512], ps[2 * mi][:], Act.Identity,
                                bias=bqf[:, m0 + mi:m0 + mi + 1], scale=1.0)
                            nc.vector.tensor_scalar(
                                out=acc_m[:, 512:1024], in0=ps[2 * mi + 1][:],
                                scalar1=bqf[:, m0 + mi:m0 + mi + 1],
                                scalar2=None, op0=Alu.add)
                        else:
                            for n in range(2):
                                nc.scalar.activation(
                                    acc_m[:, 512 * n:512 * (n + 1)],
                                    ps[2 * mi + n][:], Act.Identity,
                                    bias=bqf[:, m0 + mi:m0 + mi + 1],
                                    scale=1.0)
                        if m0 + mi == MT - 1:
                            # the final block's reduce is on the critical
                            # path into the output AllReduce - split it so
                            # each half chases its PSUM copy
                            for n in (1, 0):
                                rt = stat.tile([128, 1], f32, tag="accr")
                                nc.vector.tensor_reduce(
                                    rt[:], acc_m[:, 512 * n:512 * (n + 1)],
                                    axis=mybir.AxisListType.X,
                                    op=Alu.max, apply_absolute_value=True)
                                nc.vector.tensor_max(macc[:], macc[:], rt[:])
                        else:
                            rt = stat.tile([128, 1], f32, tag="accr")
                            nc.vector.tensor_reduce(
                                rt[:], acc_m[:], axis=mybir.AxisListType.X,
                                op=Alu.max, apply_absolute_value=True)
                            if m0 + mi == 0:
                                nc.vector.tensor_copy(macc[:], rt[:])
                            else:
                                nc.vector.tensor_max(macc[:], macc[:], rt[:])
                        accs.append(acc_m)
                    m0 += gsz
            xq_ctx.__exit__(None, None, None)

            # ---------------- AR2 + output scalars ----------------
            par2 = pers.tile([128, 1], f32, tag="par2")
            nc.gpsimd.partition_all_reduce(
                par2[:], macc[:], channels=128,
                reduce_op=bass_isa.ReduceOp.max)
            cin2 = dram.tile([1, 8], f32, tag="cin2")
            cout2 = dram.tile([1, 8], f32, tag="cout2")
            nc.sync.dma_start(cin2[0:1, 0:1], par2[0:1, 0:1])
            all_reduce_max(cin2[0:1, 0:4], cout2[0:1, 0:4])
            gm2 = pers.tile([1, 8], f32, tag="gm2")
            nc.sync.dma_start(gm2[:], cout2[:])

            # rq = out_scale/accum_scale = 127.5/gmax; ios = gmax/(127.5*asc)
            rg = pers.tile([1, 1], f32, tag="rg")
            nc.vector.reciprocal(rg[:], gm2[0:1, 0:1])
            scal2 = pers.tile([1, 4], f32, tag="scal2")
            nc.vector.tensor_scalar(out=scal2[0:1, 0:1], in0=rg[:],
                                    scalar1=127.5, scalar2=None,
                                    op0=Alu.mult)                     # rq
            nc.vector.tensor_mul(scal2[0:1, 1:2], gm2[0:1, 0:1], c2[:])  # ios
            scb2 = pers.tile([128, 4], f32, tag="scb2")
            nc.gpsimd.partition_broadcast(scb2[:], scal2[:], channels=128)

            # -------- epilogue: requant to int8 (engine-alternated) --------
            nc.sync.dma_start(ios_d.ap()[0:1, 0:4], scal2[0:1, 0:4])
            with tc.tile_pool(name="epip", bufs=8) as epip:
                def requant_p1(m, dst):
                    if m % 2 == 0 or m == 13:        # 9 blocks on ACT
                        nc.scalar.activation(dst, accs[m][:], Act.Identity,
                                             bias=cbias[:, 0:1],
                                             scale=scb2[:, 0:1])
                    else:                            # 7 on DVE
                        nc.vector.tensor_scalar(out=dst, in0=accs[m][:],
                                                scalar1=scb2[:, 0:1],
                                                scalar2=C_MAGIC,
                                                op0=Alu.mult, op1=Alu.add)

                # blocks in pairs: one DMA per two blocks so the stream
                # isn't paced by per-DMA descriptor overhead
                for pi in range(8):
                    m = 2 * pi
                    e1d = epip.tile([128, 2, BS], f32, tag="e1d")
                    requant_p1(m, e1d[:, 0, :])
                    requant_p1(m + 1, e1d[:, 1, :])
                    e2d = epip.tile([128, 2, BS], i8, tag="e2d")
                    eng2 = nc.gpsimd if pi in (0, 3) else nc.vector
                    eng2.tensor_scalar(out=e2d[:], in0=e1d[:],
                                       scalar1=C_P127, scalar2=C_MAGIC,
                                       op0=Alu.min, op1=Alu.subtract)
                    nc.sync.dma_start(
                        out_d.ap()[128 * m:128 * (m + 2), :]
                        .rearrange("(t p) b -> p t b", p=128), e2d[:])
            acc_ctx.__exit__(None, None, None)

    nc.compile()
    return nc


def _mperm(r):
    own = list(range(NSTW * r, NSTW * (r + 1)))
    return own + [m for m in range(MT) if m not in own]


def _prep_inputs(x, W, b):
    xT = np.ascontiguousarray(x.T)      # [i, b]
    WT = np.ascontiguousarray(W.T)      # [i, o]
    bfull = np.ascontiguousarray(b.reshape(32, 128).T)  # [128, 32]
    in_maps = []
    for core in range(NCORES):
        r, c = divmod(core, GC)
        roll = c * (K // GC)
        perm = _mperm(r)
        xt = np.roll(xT[:, r * BS:(r + 1) * BS], -roll, axis=0)
        wt = np.roll(WT[:, c * OS:(c + 1) * OS], -roll, axis=0)
        wt = wt.reshape(K, MT, 128)[:, perm, :].transpose(1, 0, 2)
        gcols = [16 * c + p for p in perm]
        gcols += [j for j in range(32) if j not in gcols]
        in_maps.append({
            "xt": np.ascontiguousarray(xt),
            "wt": np.ascontiguousarray(wt),
            "bp": np.ascontiguousarray(bfull[:, gcols]),
        })
    return in_maps


def kernel(x, W, b):
    from concourse import bass_utils

    x = np.asarray(x, dtype=np.float32)
    W = np.asarray(W, dtype=np.float32)
    b = np.asarray(b, dtype=np.float32)
    assert x.shape == (B, K) and W.shape == (O, K) and b.shape == (O,)

    if "nc" not in _CACHE:
        _CACHE["nc"] = _build_nc()
    nc = _CACHE["nc"]

    in_maps = _prep_inputs(x, W, b)
    res = bass_utils.run_bass_kernel_spmd(
        nc, in_maps, core_ids=list(range(NCORES)))
    _CACHE["last_results"] = res

    full = np.empty((B, O), dtype=np.float32)
    ios = np.float32(res.results[0]["ios"][0, 1])   # 1/out_scale
    for core in range(NCORES):
        r, c = divmod(core, GC)
        perm = _mperm(r)
        blk = res.results[core]["out"]          # [OS, BS] = [o, b] int8
        deq = blk.astype(np.float32) * ios      # same f32 mult the ref does
        for mp in range(MT):
            g = 16 * c + perm[mp]
            full[r * BS:(r + 1) * BS, 128 * g:128 * (g + 1)] = \
                deq[128 * mp:128 * (mp + 1), :].T
    return full
